# revision 1
# baseline (speedup 1.0000x reference)
"""GQA causal attention (B=2,S=2048,HID=2048,H=16,KVH=4,D=128) on 8 trn2 cores.

Sharding: core = b*4 + g  (b: batch, g: head-group of 4 Q heads + 1 KV head).
Per-core kernel computes q/k/v projections (+RoPE), causal softmax attention
for its 4 heads, and a partial output projection; host sums the 4 partials
per batch.

Layout strategy (all matmuls bf16 with fp32 PSUM accumulation):
  - hiddenT [HID, S] per batch; weights pre-transposed on host.
  - Projections produce qT/kT/vT [dims, S]; RoPE applied in qT layout
    (rotate-half = partition-shifted copy, validated on HW).
  - scoresT[sj, si] = kT_slice.T @ qT  -> exp (scale folded in, no max-sub:
    scores are O(+-10) so fp32 exp is safe) -> bf16 expT in SBUF.
  - AV: lhsT = expT[sj, si128], rhs = v_ext[sj, 129] where col 128 is ones
    -> psum[si, 0:128] = unnormalized attn out, psum[si, 128] = softmax denom.
  - normalize via reciprocal + tensor_scalar, PE-transpose x -> xT for the
    output projection out_p[s, :] = xT_slices.T @ woT.
"""

import math
import numpy as np
import ml_dtypes

B, S, HID = 2, 2048, 2048
H, KVH, D = 16, 4, 128
GROUPS = 4            # head groups == KV heads
HD_PER_G = 4          # query heads per group
N_CORES = 8
P = 128
HIDC = HID // P       # 16 hid chunks
SC = S // 512         # 4 s-chunks of 512
SB = S // P           # 16 s-blocks of 128

BF16 = ml_dtypes.bfloat16
_CACHE = {}


def build_nc(repeat=1):
    import concourse.bass as bass
    import concourse.tile as tile
    from concourse import bacc, mybir

    f32 = mybir.dt.float32
    bf16 = mybir.dt.bfloat16

    nc = bacc.Bacc("TRN2", target_bir_lowering=False, debug=False,
                   num_devices=N_CORES)

    hT = nc.dram_tensor("hT", [HID, S], bf16, kind="ExternalInput").ap()
    wqT = nc.dram_tensor("wqT", [HID, HD_PER_G * D], bf16, kind="ExternalInput").ap()
    wkT = nc.dram_tensor("wkT", [HID, D], bf16, kind="ExternalInput").ap()
    wvT = nc.dram_tensor("wvT", [HID, D], bf16, kind="ExternalInput").ap()
    woT = nc.dram_tensor("woT", [HD_PER_G * D, HID], bf16, kind="ExternalInput").ap()
    cosT = nc.dram_tensor("cosT", [D, S], f32, kind="ExternalInput").ap()
    sinT = nc.dram_tensor("sinT", [D, S], f32, kind="ExternalInput").ap()
    maskT = nc.dram_tensor("maskT", [P, P], bf16, kind="ExternalInput").ap()
    ident = nc.dram_tensor("ident", [P, P], bf16, kind="ExternalInput").ap()
    out = nc.dram_tensor("out", [S, HID], f32, kind="ExternalOutput").ap()

    inv_sqrt_d = 1.0 / math.sqrt(D)

    with tile.TileContext(nc) as tc:
        with (
            tc.tile_pool(name="consts", bufs=1) as consts,
            tc.tile_pool(name="persist", bufs=1) as persist,
            tc.tile_pool(name="hpool", bufs=2) as hpool,
            tc.tile_pool(name="rope", bufs=4) as rope,
            tc.tile_pool(name="expp", bufs=18) as expp,
            tc.tile_pool(name="small", bufs=4) as small,
            tc.tile_pool(name="outp", bufs=2) as outp,
            tc.tile_pool(name="ps512", bufs=3, space="PSUM") as ps512,
            tc.tile_pool(name="psav", bufs=2, space="PSUM") as psav,
            tc.tile_pool(name="psx", bufs=2, space="PSUM") as psx,
        ):
            # ---- constant loads --------------------------------------
            wqT_sb = consts.tile([P, HIDC, HD_PER_G * D], bf16)
            nc.sync.dma_start(out=wqT_sb, in_=wqT.rearrange("(c p) d -> p c d", p=P))
            wkT_sb = consts.tile([P, HIDC, D], bf16)
            nc.sync.dma_start(out=wkT_sb, in_=wkT.rearrange("(c p) d -> p c d", p=P))
            wvT_sb = consts.tile([P, HIDC, D], bf16)
            nc.sync.dma_start(out=wvT_sb, in_=wvT.rearrange("(c p) d -> p c d", p=P))
            woT_sb = consts.tile([P, HD_PER_G, HID], bf16)
            nc.sync.dma_start(out=woT_sb, in_=woT.rearrange("(m p) h -> p m h", p=P))
            cosT_sb = consts.tile([P, S], f32)
            nc.sync.dma_start(out=cosT_sb, in_=cosT)
            sinT_sb = consts.tile([P, S], f32)
            nc.sync.dma_start(out=sinT_sb, in_=sinT)
            mask_sb = consts.tile([P, P], bf16)
            nc.sync.dma_start(out=mask_sb, in_=maskT)
            ident_sb = consts.tile([P, P], bf16)
            nc.sync.dma_start(out=ident_sb, in_=ident)

            # ---- persistent intermediates ----------------------------
            qrT_sb = persist.tile([P, HD_PER_G, S], bf16)   # rotated qT per head
            krT_sb = persist.tile([P, S], bf16)             # rotated kT
            vT_sb = persist.tile([P, S], bf16)              # vT (pre-transpose)
            v_ext = persist.tile([P, SB, D + 1], bf16)      # v natural + ones col
            xT_sb = persist.tile([P, HD_PER_G, S], bf16)    # attn out transposed

            nc.vector.memset(v_ext[:, :, D:D + 1], 1.0)

            def rope_chunk(ps, dst_ap, c):
                """dst = ps*cos + rot_half(ps)*sin_signed on s-chunk c."""
                sl = slice(c * 512, (c + 1) * 512)
                t1 = rope.tile([P, 512], f32, tag="t1")
                nc.vector.tensor_mul(t1, ps, cosT_sb[:, sl])
                t2 = rope.tile([P, 512], f32, tag="t2")
                nc.scalar.copy(t2[0:64, :], ps[64:128, :])
                nc.scalar.copy(t2[64:128, :], ps[0:64, :])
                nc.vector.tensor_mul(t2, t2, sinT_sb[:, sl])
                nc.vector.tensor_add(dst_ap, t1, t2)

            for _rep in range(repeat):
                # ---- projections, per s-chunk ------------------------
                for c in range(SC):
                    ssl = slice(c * 512, (c + 1) * 512)
                    h_sb = hpool.tile([P, HIDC, 512], bf16, tag="h")
                    nc.sync.dma_start(
                        out=h_sb,
                        in_=hT.rearrange("(k p) s -> p k s", p=P)[:, :, ssl])

                    for hd in range(HD_PER_G):   # Q: 4 head-blocks
                        ps = ps512.tile([P, 512], f32, tag="ps512")
                        for k in range(HIDC):
                            nc.tensor.matmul(
                                ps, wqT_sb[:, k, hd * D:(hd + 1) * D],
                                h_sb[:, k, :],
                                start=(k == 0), stop=(k == HIDC - 1))
                        rope_chunk(ps, qrT_sb[:, hd, ssl], c)

                    ps = ps512.tile([P, 512], f32, tag="ps512")
                    for k in range(HIDC):        # K
                        nc.tensor.matmul(ps, wkT_sb[:, k, :], h_sb[:, k, :],
                                         start=(k == 0), stop=(k == HIDC - 1))
                    rope_chunk(ps, krT_sb[:, ssl], c)

                    ps = ps512.tile([P, 512], f32, tag="ps512")
                    for k in range(HIDC):        # V
                        nc.tensor.matmul(ps, wvT_sb[:, k, :], h_sb[:, k, :],
                                         start=(k == 0), stop=(k == HIDC - 1))
                    nc.scalar.copy(vT_sb[:, ssl], ps)

                # ---- v: transpose to natural layout + ones column ----
                for jb in range(SB):
                    pst = psx.tile([P, P], bf16, tag="psx")
                    nc.tensor.transpose(pst, vT_sb[:, jb * P:(jb + 1) * P], ident_sb)
                    nc.scalar.copy(v_ext[:, jb, 0:D], pst)

                # ---- attention, per head / si-chunk ------------------
                for hd in range(HD_PER_G):
                    for c in range(SC):
                        n_jb = HD_PER_G * c + HD_PER_G   # sj blocks: 0..4c+3
                        exp_tiles = []
                        for jb in range(n_jb):
                            si_start = max(c * 512, jb * P)
                            n = (c + 1) * 512 - si_start
                            pss = ps512.tile([P, 512], f32, tag="ps512")
                            nc.tensor.matmul(
                                pss[:, :n],
                                krT_sb[:, jb * P:(jb + 1) * P],
                                qrT_sb[:, hd, si_start:(c + 1) * 512],
                                start=True, stop=True)
                            et = expp.tile([P, 512], bf16, tag="expT")
                            nc.scalar.activation(
                                et[:, :n], pss[:, :n],
                                func=mybir.ActivationFunctionType.Exp,
                                scale=inv_sqrt_d)
                            if jb >= HD_PER_G * c:  # diagonal block: causal mask
                                nc.vector.tensor_mul(et[:, 0:P], et[:, 0:P], mask_sb)
                            exp_tiles.append((et, si_start))

                        for m in range(4):
                            ib = HD_PER_G * c + m
                            pav = psav.tile([P, D + 1], f32, tag="psav")
                            for jb in range(ib + 1):
                                et, si_start = exp_tiles[jb]
                                off = c * 512 + m * P - si_start
                                nc.tensor.matmul(
                                    pav, et[:, off:off + P],
                                    v_ext[:, jb, :],
                                    start=(jb == 0), stop=(jb == ib))
                            rec = small.tile([P, 1], f32, tag="rec")
                            nc.vector.reciprocal(rec, pav[:, D:D + 1])
                            xt = small.tile([P, P], bf16, tag="xt")
                            nc.vector.tensor_scalar_mul(xt, pav[:, 0:D], rec)
                            pxt = psx.tile([P, P], bf16, tag="psx")
                            nc.tensor.transpose(pxt, xt, ident_sb)
                            nc.scalar.copy(xT_sb[:, hd, ib * P:(ib + 1) * P], pxt)

                # ---- output projection -------------------------------
                for sb in range(SB):
                    out_t = outp.tile([P, HID], f32, tag="out")
                    for j in range(HID // 512):
                        pso = ps512.tile([P, 512], f32, tag="ps512")
                        for m in range(HD_PER_G):
                            nc.tensor.matmul(
                                pso, xT_sb[:, m, sb * P:(sb + 1) * P],
                                woT_sb[:, m, j * 512:(j + 1) * 512],
                                start=(m == 0), stop=(m == HD_PER_G - 1))
                        nc.vector.tensor_copy(out_t[:, j * 512:(j + 1) * 512], pso)
                    nc.sync.dma_start(out=out[sb * P:(sb + 1) * P, :], in_=out_t)

    nc.compile()
    return nc


def _prep_inputs(hidden_states, cos, sin, wq, wk, wv, wo):
    """Host-side shard + layout prep. Returns in_maps for cores 0..7."""
    hidden_states = np.asarray(hidden_states, dtype=np.float32)
    cos = np.asarray(cos, dtype=np.float32)
    sin = np.asarray(sin, dtype=np.float32)
    wq = np.asarray(wq, dtype=np.float32)
    wk = np.asarray(wk, dtype=np.float32)
    wv = np.asarray(wv, dtype=np.float32)
    wo = np.asarray(wo, dtype=np.float32)

    cosT = np.ascontiguousarray(cos[:, 0, :].T)                # [D, S] f32
    sinT_full = sin[:, 0, :].T                                  # [D, S]
    sinT = np.concatenate([-sinT_full[:64], sinT_full[64:]], axis=0)
    sinT = np.ascontiguousarray(sinT.astype(np.float32))

    mask = (np.arange(P)[:, None] <= np.arange(P)[None, :]).astype(BF16)
    identity = np.eye(P, dtype=BF16)

    hTs = [np.ascontiguousarray(hidden_states[b].T).astype(BF16)
           for b in range(B)]

    in_maps = []
    for core in range(N_CORES):
        b, g = divmod(core, GROUPS)
        qsl = slice(g * HD_PER_G * D, (g + 1) * HD_PER_G * D)
        ksl = slice(g * D, (g + 1) * D)
        in_maps.append({
            "hT": hTs[b],
            "wqT": np.ascontiguousarray(wq[qsl, :].T).astype(BF16),
            "wkT": np.ascontiguousarray(wk[ksl, :].T).astype(BF16),
            "wvT": np.ascontiguousarray(wv[ksl, :].T).astype(BF16),
            "woT": np.ascontiguousarray(wo[:, qsl].T).astype(BF16),
            "cosT": cosT,
            "sinT": sinT,
            "maskT": mask,
            "ident": identity,
        })
    return in_maps


def kernel(hidden_states, cos, sin, wq, wk, wv, wo):
    from concourse.bass_utils import run_bass_kernel_spmd

    if "nc" not in _CACHE:
        _CACHE["nc"] = build_nc()
    nc = _CACHE["nc"]

    in_maps = _prep_inputs(hidden_states, cos, sin, wq, wk, wv, wo)
    res = run_bass_kernel_spmd(nc, in_maps, core_ids=list(range(N_CORES)))

    out = np.zeros((B, S, HID), dtype=np.float32)
    for core in range(N_CORES):
        b = core // GROUPS
        out[b] += res.results[core]["out"]
    return out


# revision 3
# speedup vs baseline: 3399.9945x; 3399.9945x over previous
"""GQA causal attention (B=2,S=2048,HID=2048,H=16,KVH=4,D=128) on 8 trn2 cores.

Sharding: core = b*4 + g  (b: batch, g: head-group of 4 Q heads + 1 KV head).
Per-core kernel computes q/k/v projections (+RoPE), causal softmax attention
for its 4 heads, and a partial output projection; host sums the 4 partials
per batch.

Layout strategy (all matmuls bf16 with fp32 PSUM accumulation):
  - hiddenT [HID, S] per batch; weights pre-transposed on host.
  - Projections produce qT/kT/vT [dims, S]; RoPE applied in qT layout
    (rotate-half = partition-shifted copy, validated on HW).
  - scoresT[sj, si] = kT_slice.T @ qT  -> exp (scale folded in, no max-sub:
    scores are O(+-10) so fp32 exp is safe) -> bf16 expT in SBUF.
  - AV: lhsT = expT[sj, si128], rhs = v_ext[sj, 129] where col 128 is ones
    -> psum[si, 0:128] = unnormalized attn out, psum[si, 128] = softmax denom.
  - normalize via reciprocal + tensor_scalar, PE-transpose x -> xT for the
    output projection out_p[s, :] = xT_slices.T @ woT.
"""

import math
import numpy as np
import ml_dtypes

B, S, HID = 2, 2048, 2048
H, KVH, D = 16, 4, 128
GROUPS = 4            # head groups == KV heads
HD_PER_G = 4          # query heads per group
N_CORES = 8
P = 128
HIDC = HID // P       # 16 hid chunks
SC = S // 512         # 4 s-chunks of 512
SB = S // P           # 16 s-blocks of 128

BF16 = ml_dtypes.bfloat16
_CACHE = {}


def build_nc(repeat=1, loop_n=None):
    """loop_n: if set, wrap the body in a hardware For_i loop (for timing)."""
    import contextlib
    import concourse.bass as bass
    import concourse.tile as tile
    from concourse import bacc, mybir

    f32 = mybir.dt.float32
    bf16 = mybir.dt.bfloat16

    nc = bacc.Bacc("TRN2", target_bir_lowering=False, debug=False,
                   num_devices=N_CORES)

    hT = nc.dram_tensor("hT", [HID, S], bf16, kind="ExternalInput").ap()
    wqT = nc.dram_tensor("wqT", [HID, HD_PER_G * D], bf16, kind="ExternalInput").ap()
    wkT = nc.dram_tensor("wkT", [HID, D], bf16, kind="ExternalInput").ap()
    wvT = nc.dram_tensor("wvT", [HID, D], bf16, kind="ExternalInput").ap()
    woT = nc.dram_tensor("woT", [HD_PER_G * D, HID], bf16, kind="ExternalInput").ap()
    cosT = nc.dram_tensor("cosT", [D, S], f32, kind="ExternalInput").ap()
    sinT = nc.dram_tensor("sinT", [D, S], f32, kind="ExternalInput").ap()
    maskT = nc.dram_tensor("maskT", [P, P], bf16, kind="ExternalInput").ap()
    ident = nc.dram_tensor("ident", [P, P], bf16, kind="ExternalInput").ap()
    out = nc.dram_tensor("out", [S, HID], f32, kind="ExternalOutput").ap()

    inv_sqrt_d = 1.0 / math.sqrt(D)

    with tile.TileContext(nc) as tc:
        with (
            tc.tile_pool(name="consts", bufs=1) as consts,
            tc.tile_pool(name="persist", bufs=1) as persist,
            tc.tile_pool(name="hpool", bufs=2) as hpool,
            tc.tile_pool(name="rope", bufs=4) as rope,
            tc.tile_pool(name="expp", bufs=18) as expp,
            tc.tile_pool(name="small", bufs=4) as small,
            tc.tile_pool(name="outp", bufs=2) as outp,
            tc.tile_pool(name="ps512", bufs=3, space="PSUM") as ps512,
            tc.tile_pool(name="psav", bufs=2, space="PSUM") as psav,
            tc.tile_pool(name="psx", bufs=2, space="PSUM") as psx,
        ):
            # ---- constant loads --------------------------------------
            wqT_sb = consts.tile([P, HIDC, HD_PER_G * D], bf16)
            nc.sync.dma_start(out=wqT_sb, in_=wqT.rearrange("(c p) d -> p c d", p=P))
            wkT_sb = consts.tile([P, HIDC, D], bf16)
            nc.sync.dma_start(out=wkT_sb, in_=wkT.rearrange("(c p) d -> p c d", p=P))
            wvT_sb = consts.tile([P, HIDC, D], bf16)
            nc.sync.dma_start(out=wvT_sb, in_=wvT.rearrange("(c p) d -> p c d", p=P))
            woT_sb = consts.tile([P, HD_PER_G, HID], bf16)
            nc.sync.dma_start(out=woT_sb, in_=woT.rearrange("(m p) h -> p m h", p=P))
            cosT_sb = consts.tile([P, S], f32)
            nc.sync.dma_start(out=cosT_sb, in_=cosT)
            sinT_sb = consts.tile([P, S], f32)
            nc.sync.dma_start(out=sinT_sb, in_=sinT)
            mask_sb = consts.tile([P, P], bf16)
            nc.sync.dma_start(out=mask_sb, in_=maskT)
            ident_sb = consts.tile([P, P], bf16)
            nc.sync.dma_start(out=ident_sb, in_=ident)

            # ---- persistent intermediates ----------------------------
            qrT_sb = persist.tile([P, HD_PER_G, S], bf16)   # rotated qT per head
            krT_sb = persist.tile([P, S], bf16)             # rotated kT
            vT_sb = persist.tile([P, S], bf16)              # vT (pre-transpose)
            v_ext = persist.tile([P, SB, D + 1], bf16)      # v natural + ones col
            xT_sb = persist.tile([P, HD_PER_G, S], bf16)    # attn out transposed

            nc.vector.memset(v_ext[:, :, D:D + 1], 1.0)

            def rope_chunk(ps, dst_ap, c):
                """dst = ps*cos + rot_half(ps)*sin_signed on s-chunk c."""
                sl = slice(c * 512, (c + 1) * 512)
                t1 = rope.tile([P, 512], f32, tag="t1")
                nc.vector.tensor_mul(t1, ps, cosT_sb[:, sl])
                t2 = rope.tile([P, 512], f32, tag="t2")
                nc.scalar.copy(t2[0:64, :], ps[64:128, :])
                nc.scalar.copy(t2[64:128, :], ps[0:64, :])
                nc.vector.tensor_mul(t2, t2, sinT_sb[:, sl])
                nc.vector.tensor_add(dst_ap, t1, t2)

            loop_cm = (tc.For_i(0, loop_n, 1) if loop_n is not None
                       else contextlib.nullcontext())
            with loop_cm:
              for _rep in range(repeat):
                # ---- projections, per s-chunk ------------------------
                for c in range(SC):
                    ssl = slice(c * 512, (c + 1) * 512)
                    h_sb = hpool.tile([P, HIDC, 512], bf16, tag="h")
                    nc.sync.dma_start(
                        out=h_sb,
                        in_=hT.rearrange("(k p) s -> p k s", p=P)[:, :, ssl])

                    for hd in range(HD_PER_G):   # Q: 4 head-blocks
                        ps = ps512.tile([P, 512], f32, tag="ps512")
                        for k in range(HIDC):
                            nc.tensor.matmul(
                                ps, wqT_sb[:, k, hd * D:(hd + 1) * D],
                                h_sb[:, k, :],
                                start=(k == 0), stop=(k == HIDC - 1))
                        rope_chunk(ps, qrT_sb[:, hd, ssl], c)

                    ps = ps512.tile([P, 512], f32, tag="ps512")
                    for k in range(HIDC):        # K
                        nc.tensor.matmul(ps, wkT_sb[:, k, :], h_sb[:, k, :],
                                         start=(k == 0), stop=(k == HIDC - 1))
                    rope_chunk(ps, krT_sb[:, ssl], c)

                    ps = ps512.tile([P, 512], f32, tag="ps512")
                    for k in range(HIDC):        # V
                        nc.tensor.matmul(ps, wvT_sb[:, k, :], h_sb[:, k, :],
                                         start=(k == 0), stop=(k == HIDC - 1))
                    nc.scalar.copy(vT_sb[:, ssl], ps)

                # ---- v: transpose to natural layout + ones column ----
                for jb in range(SB):
                    pst = psx.tile([P, P], bf16, tag="psx")
                    nc.tensor.transpose(pst, vT_sb[:, jb * P:(jb + 1) * P], ident_sb)
                    nc.scalar.copy(v_ext[:, jb, 0:D], pst)

                # ---- attention, per head / si-chunk ------------------
                for hd in range(HD_PER_G):
                    for c in range(SC):
                        n_jb = HD_PER_G * c + HD_PER_G   # sj blocks: 0..4c+3
                        exp_tiles = []
                        for jb in range(n_jb):
                            si_start = max(c * 512, jb * P)
                            n = (c + 1) * 512 - si_start
                            pss = ps512.tile([P, 512], f32, tag="ps512")
                            nc.tensor.matmul(
                                pss[:, :n],
                                krT_sb[:, jb * P:(jb + 1) * P],
                                qrT_sb[:, hd, si_start:(c + 1) * 512],
                                start=True, stop=True)
                            et = expp.tile([P, 512], bf16, tag="expT")
                            nc.scalar.activation(
                                et[:, :n], pss[:, :n],
                                func=mybir.ActivationFunctionType.Exp,
                                scale=inv_sqrt_d)
                            if jb >= HD_PER_G * c:  # diagonal block: causal mask
                                nc.vector.tensor_mul(et[:, 0:P], et[:, 0:P], mask_sb)
                            exp_tiles.append((et, si_start))

                        for m in range(4):
                            ib = HD_PER_G * c + m
                            pav = psav.tile([P, D + 1], f32, tag="psav")
                            for jb in range(ib + 1):
                                et, si_start = exp_tiles[jb]
                                off = c * 512 + m * P - si_start
                                nc.tensor.matmul(
                                    pav, et[:, off:off + P],
                                    v_ext[:, jb, :],
                                    start=(jb == 0), stop=(jb == ib))
                            rec = small.tile([P, 1], f32, tag="rec")
                            nc.vector.reciprocal(rec, pav[:, D:D + 1])
                            xt = small.tile([P, P], bf16, tag="xt")
                            nc.vector.tensor_scalar_mul(xt, pav[:, 0:D], rec)
                            pxt = psx.tile([P, P], bf16, tag="psx")
                            nc.tensor.transpose(pxt, xt, ident_sb)
                            nc.scalar.copy(xT_sb[:, hd, ib * P:(ib + 1) * P], pxt)

                # ---- output projection -------------------------------
                for sb in range(SB):
                    out_t = outp.tile([P, HID], f32, tag="out")
                    for j in range(HID // 512):
                        pso = ps512.tile([P, 512], f32, tag="ps512")
                        for m in range(HD_PER_G):
                            nc.tensor.matmul(
                                pso, xT_sb[:, m, sb * P:(sb + 1) * P],
                                woT_sb[:, m, j * 512:(j + 1) * 512],
                                start=(m == 0), stop=(m == HD_PER_G - 1))
                        nc.vector.tensor_copy(out_t[:, j * 512:(j + 1) * 512], pso)
                    nc.sync.dma_start(out=out[sb * P:(sb + 1) * P, :], in_=out_t)

    nc.compile()
    return nc


def _prep_inputs(hidden_states, cos, sin, wq, wk, wv, wo):
    """Host-side shard + layout prep. Returns in_maps for cores 0..7."""
    hidden_states = np.asarray(hidden_states, dtype=np.float32)
    cos = np.asarray(cos, dtype=np.float32)
    sin = np.asarray(sin, dtype=np.float32)
    wq = np.asarray(wq, dtype=np.float32)
    wk = np.asarray(wk, dtype=np.float32)
    wv = np.asarray(wv, dtype=np.float32)
    wo = np.asarray(wo, dtype=np.float32)

    cosT = np.ascontiguousarray(cos[:, 0, :].T)                # [D, S] f32
    sinT_full = sin[:, 0, :].T                                  # [D, S]
    sinT = np.concatenate([-sinT_full[:64], sinT_full[64:]], axis=0)
    sinT = np.ascontiguousarray(sinT.astype(np.float32))

    mask = (np.arange(P)[:, None] <= np.arange(P)[None, :]).astype(BF16)
    identity = np.eye(P, dtype=BF16)

    hTs = [np.ascontiguousarray(hidden_states[b].T).astype(BF16)
           for b in range(B)]

    in_maps = []
    for core in range(N_CORES):
        b, g = divmod(core, GROUPS)
        qsl = slice(g * HD_PER_G * D, (g + 1) * HD_PER_G * D)
        ksl = slice(g * D, (g + 1) * D)
        in_maps.append({
            "hT": hTs[b],
            "wqT": np.ascontiguousarray(wq[qsl, :].T).astype(BF16),
            "wkT": np.ascontiguousarray(wk[ksl, :].T).astype(BF16),
            "wvT": np.ascontiguousarray(wv[ksl, :].T).astype(BF16),
            "woT": np.ascontiguousarray(wo[:, qsl].T).astype(BF16),
            "cosT": cosT,
            "sinT": sinT,
            "maskT": mask,
            "ident": identity,
        })
    return in_maps


def kernel(hidden_states, cos, sin, wq, wk, wv, wo):
    from concourse.bass_utils import run_bass_kernel_spmd

    if "nc" not in _CACHE:
        _CACHE["nc"] = build_nc()
    nc = _CACHE["nc"]

    in_maps = _prep_inputs(hidden_states, cos, sin, wq, wk, wv, wo)
    res = run_bass_kernel_spmd(nc, in_maps, core_ids=list(range(N_CORES)))

    out = np.zeros((B, S, HID), dtype=np.float32)
    for core in range(N_CORES):
        b = core // GROUPS
        out[b] += res.results[core]["out"]
    return out
